# revision 1
# baseline (speedup 1.0000x reference)
"""Trainium2 Bass kernel for nn_BioConvolution (locally-connected conv,
stride == kernel, unshared per-location filters).

  X [64, 64, 64, 64] f32 (N, H, W, Cin), filters [1, 256, 4, 4, 64, 128],
  bias [128]  ->  out [64, 16, 16, 128] f32
  out[n, r, c, f] = relu(sum_{i,j,ch} X[n, 4r+i, 4c+j, ch]
                         * filters[0, r*16+c, i, j, ch, f] + bias[f])

Sharding: the L = 256 location axis is split over 8 NeuronCores (the
natural spatial/tensor split — weights are unshared per location, so there
is no cross-device reduction).  Core a owns patch rows {2a, 2a+1} = 32
locations, i.e. image rows [8a, 8a+8) of X and filters[0, 32a:32a+32].

Per-location GEMM: patches [64n x 1024K] @ filters [1024K x 128F].
Compute dtype is fp16: inputs are ~N(0,1) and 0.01*N(0,1), so fp16's
10-bit mantissa gives ~3e-4 scale-relative absmax error (measured) while
halving HBM traffic — this kernel is HBM-bandwidth-bound (~12.6 MB/core:
8.4 MB filters + 4.2 MB patches + 1 MB output).

On-device dataflow per core, pipelined in groups of 4 columns:
  1. HW DMA-transpose (xbar) loads the patch block [128 batch-rows x 4096]
     directly transposed into SBUF as patchesT tiles [128 K-rows, batch]
     (the tensor engine contracts over the partition dim, so patches must
     enter K-major; the 2-byte xbar transpose does this at DMA time).
  2. Filters stream in q-major layout (2 MB chunks, contiguous per
     partition) on the second HWDGE ring.
  3. Per location: 8 accumulating matmuls [128K, 64n]^T @ [128K, 128F]
     into PSUM + one K=1 rank-1 matmul (ones x bias) to add bias in PSUM.
  4. ReLU on ScalarE (PSUM -> SBUF row buffer), per-group output DMA on
     the SWDGE ring (fp16; upcast to f32 on host).
No collectives are needed; the host concatenates the 8 location shards.
"""
import numpy as np

N, H, W, C = 64, 64, 64, 64
FH, FW, F = 4, 4, 128
R = Cc = 16          # 16x16 patch grid
K = FH * FW * C      # 1024 contraction
NC_CORES = 8
RPC = R // NC_CORES  # patch rows per core = 2

_compiled = {}


def _host_shards(X, filters, bias, dtype):
    """Per-core input maps. Host work is sharding + layout: slice rows,
    regroup (row-pair, batch) onto SBUF partitions, cast to fp16."""
    X = np.asarray(X, np.float32)
    filters = np.asarray(filters, np.float32)
    bias = np.asarray(bias, np.float32)

    # B[r, n, c, K]: patch row r, batch n, column c, K = (i*4+j)*64+ch
    A = X.reshape(N, R, FH, Cc, FW, C)                     # n r i c j ch
    B = np.ascontiguousarray(A.transpose(1, 0, 3, 2, 4, 5)).reshape(R, N, Cc, K)
    # filters q-major per core: fl[q, c, r_local, k*128+f], K = k*128+q
    flt = filters[0].reshape(256, 8, 128, F)               # l k q f
    fl9 = flt.reshape(8, RPC, Cc, 8, 128, F)               # a r c k q f
    fl9 = fl9.transpose(0, 4, 2, 1, 3, 5)                  # a q c r k f

    in_maps = []
    for a in range(NC_CORES):
        xs = B[2 * a : 2 * a + 2].reshape(128, Cc, K).astype(dtype)
        fl = np.ascontiguousarray(fl9[a]).reshape(128, Cc, RPC, 8 * F).astype(dtype)
        in_maps.append({
            "xs": np.ascontiguousarray(xs),
            "fl": fl,
            "bias": bias.reshape(1, F).astype(dtype),
        })
    return in_maps


def _build(n_iters=1):
    import concourse.mybir as mybir
    import concourse.tile as tile
    from concourse import bacc

    dtype = mybir.dt.float16
    gcols, flchunk = 4, 2
    nc = bacc.Bacc("TRN2", target_bir_lowering=False, debug=False,
                   num_devices=NC_CORES)
    xs_d = nc.dram_tensor("xs", [128, Cc, K], dtype, kind="ExternalInput").ap()
    fl_d = nc.dram_tensor("fl", [128, Cc, RPC, 8 * F], dtype,
                          kind="ExternalInput").ap()
    bias_d = nc.dram_tensor("bias", [1, F], dtype, kind="ExternalInput").ap()
    out_d = nc.dram_tensor("out", [N, 32 * F], dtype, kind="ExternalOutput").ap()
    relu = mybir.ActivationFunctionType.Relu

    with tile.TileContext(nc) as tc:
        with (
            tc.tile_pool(name="const", bufs=1) as const_pool,
            tc.tile_pool(name="pt", bufs=2) as pt_pool,
            tc.tile_pool(name="fl", bufs=6) as fl_pool,
            tc.tile_pool(name="ps", bufs=8, space="PSUM") as ps_pool,
            tc.tile_pool(name="orow", bufs=2) as orow_pool,
        ):
            ones_t = const_pool.tile([1, N], dtype, tag="ones")
            nc.vector.memset(ones_t[:], 1.0)
            bias_t = const_pool.tile([1, F], dtype, tag="bias")
            nc.scalar.dma_start(bias_t[:], bias_d[:])

            for _ in range(n_iters):
                orow = [orow_pool.tile([N, Cc * F], dtype,
                                       name=f"orow{r}", tag=f"orow{r}")
                        for r in range(RPC)]
                for c0 in range(0, Cc, gcols):
                    # patch block: one xbar-transposed DMA -> [q, (col k), e]
                    pt_sb = pt_pool.tile([128, gcols * 8 * 128], dtype, tag="pt")
                    nc.sync.dma_start(
                        pt_sb[:].rearrange("q (ck e) -> q ck e", e=128),
                        xs_d[:, c0 : c0 + gcols, :],
                        transpose=True,
                    )
                    fl_sbs = {}
                    for f0 in range(0, gcols, flchunk):
                        fl_sb = fl_pool.tile([128, flchunk * RPC * 8 * F],
                                             dtype, tag="fl")
                        nc.scalar.dma_start(
                            fl_sb[:], fl_d[:, c0 + f0 : c0 + f0 + flchunk])
                        for i in range(flchunk):
                            for r in range(RPC):
                                fl_sbs[(f0 + i, r)] = fl_sb[
                                    :, ((i * RPC + r) * 8) * F
                                    : ((i * RPC + r) * 8 + 8) * F]
                    for ci in range(gcols):
                        c = c0 + ci
                        for r in range(RPC):
                            ps = ps_pool.tile([N, F], mybir.dt.float32, tag="ps")
                            for k in range(8):
                                nc.tensor.matmul(
                                    ps[:],
                                    lhsT=pt_sb[:, (ci * 8 + k) * 128 + r * 64
                                               : (ci * 8 + k) * 128 + r * 64 + 64],
                                    rhs=fl_sbs[(ci, r)][:, k * F : (k + 1) * F],
                                    start=(k == 0), stop=False,
                                )
                            nc.tensor.matmul(ps[:], lhsT=ones_t[0:1, :],
                                             rhs=bias_t[0:1, :],
                                             start=False, stop=True)
                            nc.scalar.activation(orow[r][:, c * F : (c + 1) * F],
                                                 ps[:], relu)
                    for r in range(RPC):
                        nc.gpsimd.dma_start(
                            out_d[:, (r * Cc + c0) * F : (r * Cc + c0 + gcols) * F],
                            orow[r][:, c0 * F : (c0 + gcols) * F])
    nc.compile()
    return nc


def kernel(X, filters, bias):
    from concourse.bass_utils import run_bass_kernel_spmd

    assert X.shape == (N, H, W, C), X.shape
    assert filters.shape == (1, R * Cc, FH, FW, C, F), filters.shape
    assert bias.shape == (F,), bias.shape

    in_maps = _host_shards(X, filters, bias, np.float16)
    if "nc" not in _compiled:
        _compiled["nc"] = _build(n_iters=1)
    res = run_bass_kernel_spmd(_compiled["nc"], in_maps, list(range(NC_CORES)))

    shards = [np.asarray(res.results[a]["out"], np.float32).reshape(N, 32, F)
              for a in range(NC_CORES)]
    out = np.concatenate(shards, axis=1)       # [64, 256, 128], l = 32a + r*16+c
    return np.ascontiguousarray(out.reshape(N, R, Cc, F)).astype(np.float32)
